# revision 1
# baseline (speedup 1.0000x reference)
"""Trainium2 Bass kernel for a transformer decoder layer (self-attn + cross-attn + FFN).

Sharding: 8 cores = 4 batches x 2 halves. Core h of a batch pair owns the
interleaved query tiles {h, h+2, ..., h+14} (causal load balance) and computes
K/V projections for the contiguous token half [h*1024, (h+1)*1024); the halves
are exchanged with the pair partner via intra-chip AllGather, which hides
under ~100us of projection PE work. Causal masking is data-driven (per-core
global index vectors) so the single SPMD program is uniform across cores.

Layouts: activations for matmuls are kept transposed ([d, tokens], d on
partitions) so projections, scores (K^T Q) and attn@V (E^T V) all contract
along partitions; the only on-chip transposes are the two residual-stream
transposes (y1, y2) on the PE. Softmax runs without max-subtraction (scores
are bounded ~|2.6| at this problem's scale); the denominator comes from an
all-ones column appended to V.
"""

from contextlib import ExitStack

import numpy as np

import concourse.bass as bass
import concourse.mybir as mybir
import concourse.tile as tile
from concourse import bacc
from concourse.bass_utils import run_bass_kernel_spmd
from concourse.masks import make_identity

f32 = mybir.dt.float32
f16 = mybir.dt.float16

P = 128
D = 1024          # d_model
S = 2048          # kv sequence length
NQ = 1024         # query tokens per core
DFF = 4096
DTI = D // P      # 8 d-model partition tiles
KTI = S // P      # 16 kv token tiles
QTI = NQ // P     # 8 query tiles
FTI = DFF // P    # 32 d_ff tiles
NCH = NQ // 512   # 2 query chunks of 512
ACT = mybir.ActivationFunctionType
ALU = mybir.AluOpType
N_CORES = 8
SCALE = 1.0 / 32.0  # 1/sqrt(D)
PAIRS = [[0, 1], [2, 3], [4, 5], [6, 7]]


def _self_visible(t, c):
    """Queries are interleaved: core h owns global q-tiles {h, h+2, ...}, so
    local q-tile u is global tile 2u+h <= 2u+1; chunk c (tiles 4c..4c+3) can
    see k-tile t iff t <= 2(4c+3)+1, i.e. t < 8(c+1)."""
    return t < 8 * (c + 1)


def _self_needs_mask(t, c):
    # t < 8c is fully visible for every tile of chunk c on every core
    return t >= 8 * c


def build_nc(reps=1, use_gather=True):
    nc = bacc.Bacc("TRN2", target_bir_lowering=False, debug=False,
                   num_devices=N_CORES)

    def dp(name, shape, dt, out=False):
        return nc.declare_dram_parameter(name, shape, dt, isOutput=out)

    yqT_d = dp("yqT", [D, NQ], f16)
    ykvhT_d = dp("ykvhT", [D, NQ], f16)
    zhT_d = dp("zhT", [D, NQ], f16)
    ykvT_d = dp("ykvT", [D, S], f16)
    zT_d = dp("zT", [D, S], f16)
    yres_d = dp("yres", [NQ, D], f16)
    qg_d = dp("qg", [NQ], f32)
    kg_d = dp("kg", [S], f32)
    w_d = {n: dp(n, [D, D], f16)
           for n in ["wq1", "wk1", "wv1", "wq2", "wk2", "wv2"]}
    wf1_d = dp("wf1", [D, DFF], f16)
    wf2_d = dp("wf2", [DFF, D], f16)
    bf1_d = dp("bf1", [P, FTI], f32)
    vec_d = {n: dp(n, [D], f32)
             for n in ["bf2", "g1", "be1", "g2", "be2", "g3", "be3"]}
    out_d = dp("out", [NQ, D], f32, out=True)

    def bc(ap):  # broadcast a [n] dram vector across 128 partitions
        return bass.AP(tensor=ap.tensor, offset=ap.offset,
                       ap=[[0, P]] + [list(x) for x in ap.ap])

    with tile.TileContext(nc) as tc, ExitStack() as top:
        const = top.enter_context(tc.tile_pool(name="const", bufs=1))
        dramp = top.enter_context(tc.tile_pool(name="dramp", bufs=1,
                                               space="DRAM"))
        ident = const.tile([P, P], f16, name="ident", tag="ident")
        make_identity(nc, ident)
        kidx = const.tile([P, KTI], f32, name="kidx", tag="kidx")
        nc.sync.dma_start(out=kidx, in_=kg_d.ap().rearrange("(n p) -> p n", p=P))
        qgb = const.tile([P, NQ], f32, name="qgb", tag="qgb")
        nc.sync.dma_start(out=qgb, in_=bc(qg_d.ap()))
        eps = const.tile([P, 1], f32, name="eps", tag="eps")
        nc.vector.memset(eps, 1e-5)
        bf1_sb = const.tile([P, FTI], f32, name="bf1_sb", tag="bf1")
        nc.sync.dma_start(out=bf1_sb, in_=bf1_d.ap())
        ones1 = const.tile([P, 1], f16, name="ones1", tag="ones1")
        nc.vector.memset(ones1, 1.0)

        def load_vec_bcast(pool, name):
            t = pool.tile([P, D], f32, name=f"{name}_sb", tag=f"vb_{name}")
            nc.sync.dma_start(out=t, in_=bc(vec_d[name].ap()))
            return t

        def load_weight(pool, dram, wname):
            tiles = []
            for j in range(DTI):
                t = pool.tile([P, D], f16, name=f"{wname}{j}", tag=f"w{j}")
                nc.sync.dma_start(out=t, in_=dram.ap()[j * P:(j + 1) * P, :])
                tiles.append(t)
            return tiles

        def layer_norm(lnp, x, gb, bb, out):
            """out = (x - mean) * rsqrt(var + eps) * gb + bb, per row."""
            stats = lnp.tile([P, 2, 6], f32, name="stats", tag="stats")
            nc.vector.bn_stats(out=stats[:, 0, :], in_=x[:, 0:512])
            nc.vector.bn_stats(out=stats[:, 1, :], in_=x[:, 512:1024])
            mv = lnp.tile([P, 2], f32, name="mv", tag="mv")
            nc.vector.bn_aggr(out=mv, in_=stats)
            std = lnp.tile([P, 1], f32, name="std", tag="std")
            nc.scalar.activation(out=std, in_=mv[:, 1:2], func=ACT.Sqrt,
                                 bias=eps, scale=1.0)
            rstd = lnp.tile([P, 1], f32, name="rstd", tag="rstd")
            nc.vector.reciprocal(rstd, std)
            tmp = lnp.tile([P, D], f32, name="lntmp", tag="lntmp", bufs=2)
            nc.vector.tensor_scalar(out=tmp, in0=x, scalar1=mv[:, 0:1],
                                    scalar2=rstd, op0=ALU.subtract,
                                    op1=ALU.mult)
            nc.vector.tensor_mul(out=tmp, in0=tmp, in1=gb)
            nc.vector.tensor_add(out=out, in0=tmp, in1=bb)

        def project_qT(psum, wtiles, src_fn, qT, nchunks):
            # qT[i][:, c*512:...] = (W.T @ src), contraction over d_in tiles
            for i in range(DTI):
                for c in range(nchunks):
                    ps = psum.tile([P, 512], f32, name="ps_proj", tag="ps_proj")
                    for j in range(DTI):
                        nc.tensor.matmul(ps, lhsT=wtiles[j][:, i * P:(i + 1) * P],
                                         rhs=src_fn(j, c),
                                         start=(j == 0), stop=(j == DTI - 1))
                    nc.scalar.copy(out=qT[i][:, c * 512:(c + 1) * 512], in_=ps)

        def kv_half_gather(psum, stgp, stage, srch, wk_t, wv_t,
                           kT=None, v=None):
            """Project K/V for this core's kv token half and AllGather within
            the batch pair. Returns the gathered DRAM tiles (k: [2D, NQ],
            d-major per half; v: [S, D] token-major, natural order). If kT/v
            are given, the SBUF readback DMAs are emitted right after each
            collective (higher DMA priority than end-of-stage)."""
            k_in = dramp.tile([D, NQ], f16, name=f"k_in{stage}",
                              tag=f"k_in{stage}")
            k_out = dramp.tile([2 * D, NQ], f16, name=f"k_out{stage}",
                               tag=f"k_out{stage}")
            v_in = dramp.tile([NQ, D], f16, name=f"v_in{stage}",
                              tag=f"v_in{stage}")
            v_out = dramp.tile([S, D], f16, name=f"v_out{stage}",
                               tag=f"v_out{stage}")
            for i in range(DTI):
                kst = stgp.tile([P, NQ], f16, name="kst", tag=f"kst{stage}")
                for ck in range(NQ // 512):
                    ps = psum.tile([P, 512], f32, name="ps_kh", tag="ps_proj")
                    for j in range(DTI):
                        nc.tensor.matmul(
                            ps, lhsT=wk_t[j][:, i * P:(i + 1) * P],
                            rhs=srch[j][:, ck * 512:(ck + 1) * 512],
                            start=(j == 0), stop=(j == DTI - 1))
                    nc.vector.tensor_copy(
                        out=kst[:, ck * 512:(ck + 1) * 512], in_=ps)
                nc.sync.dma_start(out=k_in[i * P:(i + 1) * P, :], in_=kst)
            nc.gpsimd.collective_compute(
                "AllGather", ALU.bypass, replica_groups=PAIRS,
                ins=[k_in.opt()], outs=[k_out.opt()])
            if kT is not None:
                for i in range(DTI):
                    for g in range(2):
                        nc.sync.dma_start(
                            out=kT[i][:, g * NQ:(g + 1) * NQ],
                            in_=k_out[g * D + i * P:g * D + (i + 1) * P, :])
            for t in range(NQ // P):
                vst = stgp.tile([P, D], f16, name="vst", tag=f"vst{stage}")
                for n in range(2):
                    ps = psum.tile([P, 512], f32, name="ps_vh", tag="ps_proj")
                    for j in range(DTI):
                        nc.tensor.matmul(
                            ps, lhsT=srch[j][:, t * P:(t + 1) * P],
                            rhs=wv_t[j][:, n * 512:(n + 1) * 512],
                            start=(j == 0), stop=(j == DTI - 1))
                    nc.scalar.copy(out=vst[:, n * 512:(n + 1) * 512], in_=ps)
                nc.sync.dma_start(out=v_in[t * P:(t + 1) * P, :], in_=vst)
            nc.gpsimd.collective_compute(
                "AllGather", ALU.bypass, replica_groups=PAIRS,
                ins=[v_in.opt()], outs=[v_out.opt()])
            if v is not None:
                for t in range(KTI):
                    nc.sync.dma_start(out=v[t][:, 0:1024],
                                      in_=v_out[t * P:(t + 1) * P, :])
            return k_out, v_out

        def kv_readback(k_out, v_out, kT, v):
            for i in range(DTI):
                for g in range(2):
                    nc.sync.dma_start(
                        out=kT[i][:, g * NQ:(g + 1) * NQ],
                        in_=k_out[g * D + i * P:g * D + (i + 1) * P, :])
            for t in range(KTI):
                nc.sync.dma_start(out=v[t][:, 0:1024],
                                  in_=v_out[t * P:(t + 1) * P, :])

        def attention(stk, tagp, qT, kT, v, resid, gb, bb, y_out, masked):
            """scoresT = K^T Q per block -> exp -> (mask) -> out = E^T V;
            denominators batched via a ones-vector matmul per chunk."""
            psum_s = stk.enter_context(tc.tile_pool(name=f"{tagp}psum_s",
                                                    bufs=3, space="PSUM"))
            psum_o = stk.enter_context(tc.tile_pool(name=f"{tagp}psum_o",
                                                    bufs=2, space="PSUM"))
            psum_d = stk.enter_context(tc.tile_pool(name=f"{tagp}psum_d",
                                                    bufs=1, space="PSUM"))
            expp = stk.enter_context(tc.tile_pool(name=f"{tagp}expp", bufs=2))
            maskp = stk.enter_context(tc.tile_pool(name=f"{tagp}maskp",
                                                   bufs=2))
            lnp = stk.enter_context(tc.tile_pool(name=f"{tagp}lnp", bufs=4))
            for c in range(NCH):
                qsl = slice(c * 512, (c + 1) * 512)
                vis = [t for t in range(KTI)
                       if not masked or _self_visible(t, c)]
                etiles = {}
                for t in vis:
                    ps = psum_s.tile([P, 512], f32, name="ps_s", tag="ps_s")
                    for j in range(DTI):
                        nc.tensor.matmul(ps, lhsT=kT[j][:, t * P:(t + 1) * P],
                                         rhs=qT[j][:, qsl],
                                         start=(j == 0), stop=(j == DTI - 1))
                    e = expp.tile([P, 512], f16, name="e", tag=f"e{t}")
                    nc.scalar.activation(out=e, in_=ps, func=ACT.Exp,
                                         scale=SCALE)
                    if masked and _self_needs_mask(t, c):
                        m = maskp.tile([P, 512], f16, name="m", tag="mask")
                        nc.vector.tensor_scalar(out=m, in0=qgb[:, qsl],
                                                scalar1=kidx[:, t:t + 1],
                                                scalar2=None, op0=ALU.is_ge)
                        nc.vector.tensor_mul(out=e, in0=e, in1=m)
                    etiles[t] = e
                # denominators for the whole chunk: [1, 512] = ones^T @ E
                pd = psum_d.tile([1, 512], f32, name="pd", tag="pd")
                for idx, t in enumerate(vis):
                    nc.tensor.matmul(pd, lhsT=ones1, rhs=etiles[t],
                                     start=(idx == 0),
                                     stop=(idx == len(vis) - 1))
                dsb = lnp.tile([1, 512], f32, name="dsb", tag="dsb")
                nc.scalar.copy(out=dsb, in_=pd)
                dscr = dramp.tile([512], f32, name="dscr",
                                  tag=f"{tagp}dscr{c}")
                nc.sync.dma_start(out=dscr, in_=dsb)
                dT = lnp.tile([P, 4], f32, name="dT", tag="dT")
                nc.sync.dma_start(
                    out=dT, in_=dscr.rearrange("(a p) -> p a", p=P))
                recT = lnp.tile([P, 4], f32, name="recT", tag="recT")
                nc.vector.reciprocal(recT, dT)
                for u4 in range(4):
                    u = c * 4 + u4
                    # causal: local q-tile u only sees k-tiles t <= 2u+1
                    vis_u = [t for t in vis if not masked or t <= 2 * u + 1]
                    po = psum_o.tile([P, 1024], f32, name="po", tag="po")
                    for idx, t in enumerate(vis_u):
                        st, sp = idx == 0, idx == len(vis_u) - 1
                        lhsT = etiles[t][:, u4 * P:(u4 + 1) * P]
                        nc.tensor.matmul(po[:, 0:512], lhsT=lhsT,
                                         rhs=v[t][:, 0:512], start=st, stop=sp)
                        nc.tensor.matmul(po[:, 512:1024], lhsT=lhsT,
                                         rhs=v[t][:, 512:1024], start=st,
                                         stop=sp)
                    xr = lnp.tile([P, D], f32, name="xr", tag="xr",
                                  bufs=2)
                    # split the psum-drain between ACT and DVE
                    nc.scalar.activation(out=xr[:, 0:512], in_=po[:, 0:512],
                                         func=ACT.Copy,
                                         scale=recT[:, u4:u4 + 1])
                    nc.vector.tensor_scalar(out=xr[:, 512:1024],
                                            in0=po[:, 512:1024],
                                            scalar1=recT[:, u4:u4 + 1],
                                            scalar2=None, op0=ALU.mult)
                    nc.vector.tensor_add(out=xr, in0=xr, in1=resid[u])
                    layer_norm(lnp, xr, gb, bb, y_out[u])

        def transpose_qd(stk, y_h, y_Tc):
            # y_h[u]: [128q, 1024d] f16  ->  y_Tc[i][c]: [128d, 512q] f16
            psum_t = stk.enter_context(tc.tile_pool(name="psum_t", bufs=4,
                                                    space="PSUM"))
            for c in range(NCH):
                for i in range(DTI):
                    for u4 in range(4):
                        u = c * 4 + u4
                        pt = psum_t.tile([P, P], f16, name="pt", tag="pt")
                        nc.tensor.transpose(
                            pt, in_=y_h[u][:, i * P:(i + 1) * P],
                            identity=ident)
                        nc.scalar.copy(
                            out=y_Tc[i][c][:, u4 * P:(u4 + 1) * P], in_=pt)

        def emit_pass(pfx):
            # ------------ pools with cross-stage lifetimes ------------
            qkvp = tc.alloc_tile_pool(name=f"{pfx}qkvp", bufs=1)
            y1p = tc.alloc_tile_pool(name=f"{pfx}y1p", bufs=1, side="right")
            y1h = [y1p.tile([P, D], f16, name=f"y1h{u}", tag=f"y1h{u}")
                   for u in range(QTI)]

            # ===== stage A: projections (+ KV pair exchange) =====
            # PE order: K1h, V1h, Q1, K2h, V2h -- each collective then has
            # >=100us of projection work to hide under before its consumer.
            k2_out = v2_out = None
            with ExitStack() as stA:
                kvp = stA.enter_context(tc.tile_pool(name=f"{pfx}kvp", bufs=1))
                wp = stA.enter_context(tc.tile_pool(name=f"{pfx}wp", bufs=2))
                stgp = stA.enter_context(tc.tile_pool(name=f"{pfx}stgp",
                                                      bufs=3))
                psum_a = stA.enter_context(tc.tile_pool(name=f"{pfx}psum_a",
                                                        bufs=4, space="PSUM"))
                yq = [kvp.tile([P, NQ], f16, name=f"yq{j}", tag=f"yq{j}")
                      for j in range(DTI)]
                qT = [qkvp.tile([P, NQ], f16, name=f"qT{i}", tag=f"qT{i}")
                      for i in range(DTI)]
                kT = [qkvp.tile([P, S], f16, name=f"kT{i}", tag=f"kT{i}")
                      for i in range(DTI)]
                v = [qkvp.tile([P, 1024], f16, name=f"v{t}", tag=f"v{t}")
                     for t in range(KTI)]

                if use_gather:
                    ykvh = [kvp.tile([P, NQ], f16, name=f"ykvh{j}",
                                     tag=f"kv{j}") for j in range(DTI)]
                    wk = []
                    for j in range(DTI):
                        t = wp.tile([P, D], f16, name=f"wk1{j}", tag=f"w{j}")
                        nc.sync.dma_start(
                            out=t, in_=w_d["wk1"].ap()[j * P:(j + 1) * P, :])
                        wk.append(t)
                        nc.sync.dma_start(
                            out=ykvh[j],
                            in_=ykvhT_d.ap()[j * P:(j + 1) * P, :])
                    wv = load_weight(wp, w_d["wv1"], "wv1")
                    wq = load_weight(wp, w_d["wq1"], "wq1")
                    for j in range(DTI):
                        nc.sync.dma_start(
                            out=yq[j], in_=yqT_d.ap()[j * P:(j + 1) * P, :])
                    kv_half_gather(psum_a, stgp, f"{pfx}a", ykvh, wk, wv,
                                   kT=kT, v=v)
                    project_qT(psum_a, wq,
                               lambda j, c: yq[j][:, c * 512:(c + 1) * 512],
                               qT, NCH)
                    # cross-attn KV halves: zh reuses the ykvh slots
                    zh = [kvp.tile([P, NQ], f16, name=f"zh{j}", tag=f"kv{j}")
                          for j in range(DTI)]
                    wk2 = load_weight(wp, w_d["wk2"], "wk2")
                    for j in range(DTI):
                        nc.sync.dma_start(
                            out=zh[j], in_=zhT_d.ap()[j * P:(j + 1) * P, :])
                    wv2 = load_weight(wp, w_d["wv2"], "wv2")
                    k2_out, v2_out = kv_half_gather(psum_a, stgp, f"{pfx}c",
                                                    zh, wk2, wv2)
                else:
                    # local full-KV compute, no collectives
                    ykv = [kvp.tile([P, S], f16, name=f"ykv{j}",
                                    tag=f"kvf{j}") for j in range(DTI)]
                    wk = load_weight(wp, w_d["wk1"], "wk1")
                    for j in range(DTI):
                        nc.sync.dma_start(
                            out=ykv[j], in_=ykvT_d.ap()[j * P:(j + 1) * P, :])
                    wv = load_weight(wp, w_d["wv1"], "wv1")
                    wq = load_weight(wp, w_d["wq1"], "wq1")
                    for j in range(DTI):
                        nc.sync.dma_start(
                            out=yq[j], in_=yqT_d.ap()[j * P:(j + 1) * P, :])
                    for i in range(DTI):
                        for ck in range(S // 512):
                            ps = psum_a.tile([P, 512], f32, name="ps_k",
                                             tag="ps_proj")
                            for j in range(DTI):
                                nc.tensor.matmul(
                                    ps, lhsT=wk[j][:, i * P:(i + 1) * P],
                                    rhs=ykv[j][:, ck * 512:(ck + 1) * 512],
                                    start=(j == 0), stop=(j == DTI - 1))
                            nc.scalar.copy(
                                out=kT[i][:, ck * 512:(ck + 1) * 512], in_=ps)
                    for t in range(KTI):
                        for n in range(2):
                            ps = psum_a.tile([P, 512], f32, name="ps_v",
                                             tag="ps_proj")
                            for j in range(DTI):
                                nc.tensor.matmul(
                                    ps, lhsT=ykv[j][:, t * P:(t + 1) * P],
                                    rhs=wv[j][:, n * 512:(n + 1) * 512],
                                    start=(j == 0), stop=(j == DTI - 1))
                            nc.scalar.copy(out=v[t][:, n * 512:(n + 1) * 512],
                                           in_=ps)
                    project_qT(psum_a, wq,
                               lambda j, c: yq[j][:, c * 512:(c + 1) * 512],
                               qT, NCH)

            # ===== stage B: self-attention + LN1 =====
            with ExitStack() as stB:
                resp = stB.enter_context(tc.tile_pool(name=f"{pfx}resp",
                                                      bufs=1))
                gbp = stB.enter_context(tc.tile_pool(name=f"{pfx}gbp1",
                                                     bufs=1))
                yres = [resp.tile([P, D], f16, name=f"yres{u}", tag=f"yres{u}")
                        for u in range(QTI)]
                for u in range(QTI):
                    nc.sync.dma_start(out=yres[u],
                                      in_=yres_d.ap()[u * P:(u + 1) * P, :])
                g1b = load_vec_bcast(gbp, "g1")
                be1b = load_vec_bcast(gbp, "be1")
                attention(stB, f"{pfx}sa_", qT, kT, v, yres, g1b, be1b, y1h,
                          masked=True)
            qkvp.release()

            # transpose y1 -> y1T for cross-attn Q projection
            y1Tp = tc.alloc_tile_pool(name=f"{pfx}y1Tp", bufs=1)
            y1T = [[y1Tp.tile([P, 512], f16, name=f"y1T{i}_{c}",
                              tag=f"y1T{i}_{c}")
                    for c in range(NCH)] for i in range(DTI)]
            with ExitStack() as stB2:
                transpose_qd(stB2, y1h, y1T)

            # ===== stage C: cross-attention + LN2 =====
            qkv2p = tc.alloc_tile_pool(name=f"{pfx}qkv2p", bufs=1,
                                       side="right")
            qT2 = [qkv2p.tile([P, NQ], f16, name=f"qT2{i}", tag=f"qT2{i}")
                   for i in range(DTI)]
            kT2 = [qkv2p.tile([P, S], f16, name=f"kT2{i}", tag=f"kT2{i}")
                   for i in range(DTI)]
            v2 = [qkv2p.tile([P, 1024], f16, name=f"v2{t}", tag=f"v2{t}")
                  for t in range(KTI)]
            with ExitStack() as stC1:
                wp2 = stC1.enter_context(tc.tile_pool(name=f"{pfx}wp2",
                                                      bufs=2))
                psum_c = stC1.enter_context(tc.tile_pool(name=f"{pfx}psum_c",
                                                         bufs=4, space="PSUM"))
                if use_gather:
                    kv_readback(k2_out, v2_out, kT2, v2)
                else:
                    zp = stC1.enter_context(tc.tile_pool(name=f"{pfx}zp",
                                                         bufs=1))
                    zt = [zp.tile([P, S], f16, name=f"zt{j}", tag=f"z{j}")
                          for j in range(DTI)]
                    wk2 = load_weight(wp2, w_d["wk2"], "wk2")
                    for j in range(DTI):
                        nc.sync.dma_start(
                            out=zt[j], in_=zT_d.ap()[j * P:(j + 1) * P, :])
                    wv2 = load_weight(wp2, w_d["wv2"], "wv2")
                    for i in range(DTI):
                        for ck in range(S // 512):
                            ps = psum_c.tile([P, 512], f32, name="ps_k2",
                                             tag="ps_proj")
                            for j in range(DTI):
                                nc.tensor.matmul(
                                    ps, lhsT=wk2[j][:, i * P:(i + 1) * P],
                                    rhs=zt[j][:, ck * 512:(ck + 1) * 512],
                                    start=(j == 0), stop=(j == DTI - 1))
                            nc.scalar.copy(
                                out=kT2[i][:, ck * 512:(ck + 1) * 512],
                                in_=ps)
                    for t in range(KTI):
                        for n in range(2):
                            ps = psum_c.tile([P, 512], f32, name="ps_v2",
                                             tag="ps_proj")
                            for j in range(DTI):
                                nc.tensor.matmul(
                                    ps, lhsT=zt[j][:, t * P:(t + 1) * P],
                                    rhs=wv2[j][:, n * 512:(n + 1) * 512],
                                    start=(j == 0), stop=(j == DTI - 1))
                            nc.scalar.copy(
                                out=v2[t][:, n * 512:(n + 1) * 512], in_=ps)
                wq2 = load_weight(wp2, w_d["wq2"], "wq2")
                project_qT(psum_c, wq2, lambda j, c: y1T[j][c], qT2, NCH)
            y1Tp.release()

            y2p = tc.alloc_tile_pool(name=f"{pfx}y2p", bufs=1)
            y2h = [y2p.tile([P, D], f16, name=f"y2h{u}", tag=f"y2h{u}")
                   for u in range(QTI)]
            with ExitStack() as stC2:
                gbp2 = stC2.enter_context(tc.tile_pool(name=f"{pfx}gbp2",
                                                       bufs=1))
                g2b = load_vec_bcast(gbp2, "g2")
                be2b = load_vec_bcast(gbp2, "be2")
                attention(stC2, f"{pfx}ca_", qT2, kT2, v2, y1h, g2b, be2b,
                          y2h, masked=False)
            qkv2p.release()
            y1p.release()

            y2Tp = tc.alloc_tile_pool(name=f"{pfx}y2Tp", bufs=1)
            y2T = [[y2Tp.tile([P, 512], f16, name=f"y2T{i}_{c}",
                              tag=f"y2T{i}_{c}")
                    for c in range(NCH)] for i in range(DTI)]
            with ExitStack() as stC3:
                transpose_qd(stC3, y2h, y2T)

            # ===== stage D: FFN + LN3 + output =====
            with ExitStack() as stD:
                wf2p = stD.enter_context(tc.tile_pool(name=f"{pfx}wf2p",
                                                      bufs=1))
                wf1p = stD.enter_context(tc.tile_pool(name=f"{pfx}wf1p",
                                                      bufs=3))
                htp = stD.enter_context(tc.tile_pool(name=f"{pfx}htp",
                                                     bufs=1))
                gbp3 = stD.enter_context(tc.tile_pool(name=f"{pfx}gbp3",
                                                      bufs=1))
                outp = stD.enter_context(tc.tile_pool(name=f"{pfx}outp",
                                                      bufs=2))
                ln3p = stD.enter_context(tc.tile_pool(name=f"{pfx}ln3p",
                                                      bufs=4))
                psum_h = stD.enter_context(tc.tile_pool(name=f"{pfx}psum_h",
                                                        bufs=4, space="PSUM"))
                psum_f = stD.enter_context(tc.tile_pool(name=f"{pfx}psum_f",
                                                        bufs=2, space="PSUM"))
                wf2 = [wf2p.tile([P, D], f16, name=f"wf2_{s}", tag=f"wf2_{s}")
                       for s in range(FTI)]
                for s in range(FTI):
                    nc.sync.dma_start(out=wf2[s],
                                      in_=wf2_d.ap()[s * P:(s + 1) * P, :])
                g3b = load_vec_bcast(gbp3, "g3")
                be3b = load_vec_bcast(gbp3, "be3")
                bf2b = load_vec_bcast(gbp3, "bf2")

                for c in range(NCH):
                    hts = []
                    for s in range(FTI):
                        wt = wf1p.tile([P, DTI, P], f16, name="wf1s",
                                       tag="wf1s")
                        nc.sync.dma_start(
                            out=wt,
                            in_=wf1_d.ap()[:, s * P:(s + 1) * P].rearrange(
                                "(n p) m -> p n m", p=P))
                        ph = psum_h.tile([P, 512], f32, name="ph", tag="ph")
                        for j in range(DTI):
                            nc.tensor.matmul(ph, lhsT=wt[:, j, :],
                                             rhs=y2T[j][c],
                                             start=(j == 0),
                                             stop=(j == DTI - 1))
                        ht = htp.tile([P, 512], f16, name="ht", tag=f"ht{s}")
                        nc.scalar.activation(out=ht, in_=ph, func=ACT.Relu,
                                             bias=bf1_sb[:, s:s + 1],
                                             scale=1.0)
                        hts.append(ht)
                    for u4 in range(4):
                        u = c * 4 + u4
                        pf = psum_f.tile([P, D], f32, name="pf", tag="pf")
                        for n in range(2):
                            for s in range(FTI):
                                nc.tensor.matmul(
                                    pf[:, n * 512:(n + 1) * 512],
                                    lhsT=hts[s][:, u4 * P:(u4 + 1) * P],
                                    rhs=wf2[s][:, n * 512:(n + 1) * 512],
                                    start=(s == 0), stop=(s == FTI - 1))
                        xr = ln3p.tile([P, D], f32, name="xr3", tag="xr3",
                                       bufs=2)
                        nc.vector.tensor_add(out=xr, in0=pf, in1=bf2b)
                        nc.vector.tensor_add(out=xr, in0=xr, in1=y2h[u])
                        y3 = outp.tile([P, D], f32, name="y3", tag="y3")
                        layer_norm(ln3p, xr, g3b, be3b, y3)
                        nc.sync.dma_start(
                            out=out_d.ap()[u * P:(u + 1) * P, :], in_=y3)
            y2Tp.release()
            y2p.release()

        for rep in range(reps):
            emit_pass(f"r{rep}_" if reps > 1 else "")

    nc.compile()
    return nc


_CACHE = {}
USE_GATHER = True


def _get_nc(reps=1, use_gather=None):
    if use_gather is None:
        use_gather = USE_GATHER
    key = (reps, use_gather)
    if key not in _CACHE:
        _CACHE[key] = build_nc(reps=reps, use_gather=use_gather)
    return _CACHE[key]


def _q_indices(h):
    """Interleaved q-tile ownership: core-half h owns global tiles h, h+2, ..."""
    tiles = np.arange(h, 2 * QTI, 2)
    return (tiles[:, None] * P + np.arange(P)[None, :]).reshape(-1)


def _prep_core(c, y, Z, shared):
    b, h = c // 2, c % 2
    qi = _q_indices(h)
    yb = y[b]
    m = {
        "yqT": np.ascontiguousarray(yb[qi].T).astype(np.float16),
        "ykvhT": np.ascontiguousarray(
            yb[h * NQ:(h + 1) * NQ].T).astype(np.float16),
        "zhT": np.ascontiguousarray(
            Z[b, h * NQ:(h + 1) * NQ].T).astype(np.float16),
        "ykvT": np.ascontiguousarray(yb.T).astype(np.float16),
        "zT": np.ascontiguousarray(Z[b].T).astype(np.float16),
        "yres": yb[qi].astype(np.float16),
        "qg": qi.astype(np.float32),
        "kg": np.arange(S, dtype=np.float32),
    }
    m.update(shared)
    return m


def kernel(**inputs):
    inp = {k: np.asarray(v) for k, v in inputs.items()}
    y = inp["y"].astype(np.float32)
    Z = inp["Z"].astype(np.float32)
    shared = {
        "wq1": inp["WQ1"].astype(np.float16),
        "wk1": inp["WK1"].astype(np.float16),
        "wv1": inp["WV1"].astype(np.float16),
        "wq2": inp["WQ2"].astype(np.float16),
        "wk2": inp["WK2"].astype(np.float16),
        "wv2": inp["WV2"].astype(np.float16),
        "wf1": inp["W_ff1"].astype(np.float16),
        "wf2": inp["W_ff2"].astype(np.float16),
        "bf1": np.ascontiguousarray(
            inp["b_ff1"].astype(np.float32).reshape(FTI, P).T),
        "bf2": inp["b_ff2"].astype(np.float32),
        "g1": inp["g1"].astype(np.float32),
        "be1": inp["be1"].astype(np.float32),
        "g2": inp["g2"].astype(np.float32),
        "be2": inp["be2"].astype(np.float32),
        "g3": inp["g3"].astype(np.float32),
        "be3": inp["be3"].astype(np.float32),
    }
    in_maps = [_prep_core(c, y, Z, shared) for c in range(N_CORES)]
    res = run_bass_kernel_spmd(_get_nc(), in_maps, list(range(N_CORES)))
    out = np.zeros((4, 2048, 1024), np.float32)
    for c in range(N_CORES):
        b, h = c // 2, c % 2
        out[b, _q_indices(h)] = res.results[c]["out"]
    return out



# revision 24
# speedup vs baseline: 2.5437x; 2.5437x over previous
"""Trainium2 Bass kernel for a transformer decoder layer (self-attn +
cross-attn + FFN), fp8-e4m3 DoubleRow edition.

Sharding: 8 cores = 4 batches x 2 halves. Core h of a batch owns the
interleaved query tiles {h, h+2, ..., h+14} (causal load balance) and
computes the FULL K/V projections for its batch locally (the duplicated
projection work is cheap in fp8 DoubleRow and avoids all collectives).

All matmuls run in fp8e4m3 with MatmulPerfMode.DoubleRow (contraction
tiles processed in pairs laid out as [128, 2, N] APs). Weights are
pre-scaled by 32 on the host so their ~0.02-magnitude values sit in the
fp8 normal range; the 32x factors are folded into the exp scale, the
softmax-denominator reciprocal, and the FFN drain scales. The residual
stream, layer norms and softmax denominators stay in f16/f32.

Causal masking is tile-diagonal only: queries are interleaved so local
q-tile u's diagonal lives in k-tile pair u; one constant per-core
[128, 2, 128] mask (tril at parity h) multiplies that pair's E slice.
Denominators are per-q-tile ones-matmuls over the visible pairs, with a
DRAM round-trip to transpose [1, 512] -> [128, 4] per chunk.

Engine budget: PE ~180us of fp8 matmul; psum drains are split between
ACT and DVE; Pool (gpsimd) takes the SBUF-only mask multiplies and
layer-norm beta adds. rstd uses exp(-0.5*ln(var+eps)) so every ACT
activation comes from the single natural_log_exp_and_others table (no
LoadActFuncSet thrash).
"""

from contextlib import ExitStack

import numpy as np
import ml_dtypes

import concourse.bass as bass
import concourse.mybir as mybir
import concourse.tile as tile
from concourse import bacc
from concourse.bass_utils import run_bass_kernel_spmd
from concourse.masks import make_identity

f32 = mybir.dt.float32
f16 = mybir.dt.float16
f8 = mybir.dt.float8e4
ACT = mybir.ActivationFunctionType
ALU = mybir.AluOpType
DR = mybir.MatmulPerfMode.DoubleRow

P = 128
D = 1024          # d_model
S = 2048          # kv sequence length
NQ = 1024         # query tokens per core
DFF = 4096
DTI = D // P      # 8 d-model tiles
KTI = S // P      # 16 kv token tiles
QTI = NQ // P     # 8 query tiles
FTI = DFF // P    # 32 d_ff tiles
NDP = DTI // 2    # 4 d-model pairs
NKP = KTI // 2    # 8 kv pairs
NFP = FTI // 2    # 16 d_ff pairs
NCH = 2           # query chunks of 512
N_CORES = 8
WS = 32.0         # host-side weight scale
EXP_SCALE = 1.0 / (WS * WS * 32.0)   # psum = 1024*QK ; exp(QK/sqrt(D))


def build_nc():
    nc = bacc.Bacc("TRN2", target_bir_lowering=False, debug=False,
                   num_devices=N_CORES)

    def dp(name, shape, dt, out=False):
        return nc.declare_dram_parameter(name, shape, dt, isOutput=out)

    yq8_d = dp("yq8", [D, NQ], f8)
    ykv8_d = dp("ykv8", [D, S], f8)
    z8_d = dp("z8", [D, S], f8)
    yres_d = dp("yres", [NQ, D], f16)
    mask_d = dp("mask8", [P, 2, P], f8)
    ykv01_d = dp("ykv01", [D, 2 * P], f16)
    wv16_d = dp("wv16", [D, D], f16)
    w_d = {n: dp(n, [D, D], f8)
           for n in ["wq1", "wk1", "wv1", "wq2", "wk2", "wv2"]}
    wf1x_d = dp("wf1x", [2 * D, DFF], f8)
    wf2x_d = dp("wf2x", [(2 * FTI + 2) * P, D], f8)
    bf1_d = dp("bf1x32", [P, FTI], f32)
    vec_d = {n: dp(n, [D], f16)
             for n in ["g1", "be1", "g2", "be2", "g3", "be3"]}
    out_d = dp("out16", [NQ, D], f16, out=True)

    def bc(ap):  # broadcast a [n] dram vector across 128 partitions
        return bass.AP(tensor=ap.tensor, offset=ap.offset,
                       ap=[[0, P]] + [list(x) for x in ap.ap])

    def r3(ap):  # [n*p, m] dram -> [p, n, m]
        return ap.rearrange("(n p) m -> p n m", p=P)

    def dup2(ap):  # insert a stride-0 pair dim after the partition dim
        return bass.AP(tensor=ap.tensor, offset=ap.offset,
                       ap=[list(ap.ap[0]), [0, 2]] + [list(x)
                                                      for x in ap.ap[1:]])

    def pairstep2(t3, e0, csl):  # dim-1 entries {e0, e0+2} of a 3d tile
        a = t3[:, e0, csl]
        b = t3[:, e0 + 2, csl]
        return bass.AP(tensor=a.tensor, offset=a.offset,
                       ap=[list(a.ap[0]), [b.offset - a.offset, 2]]
                       + [list(x) for x in a.ap[1:]])

    with tile.TileContext(nc) as tc, ExitStack() as top:
        const = top.enter_context(tc.tile_pool(name="const", bufs=1))
        dramp = top.enter_context(tc.tile_pool(name="dramp", bufs=1,
                                               space="DRAM"))
        ident = const.tile([P, P], f16, name="ident", tag="ident")
        make_identity(nc, ident)
        ones8 = const.tile([P, 2, 32], f8, name="ones8", tag="ones8")
        nc.vector.memset(ones8, 1.0)
        e0t = const.tile([P, 2, P], f8, name="e0t", tag="e0t")
        nc.vector.memset(e0t, 0.0)
        nc.vector.memset(e0t[0:2, 0:1, :], WS)
        ones16 = const.tile([P, 32], f16, name="ones16", tag="ones16")
        nc.vector.memset(ones16, 1.0)
        mask8 = const.tile([P, 2, P], f8, name="mask8", tag="mask8")
        nc.sync.dma_start(out=mask8, in_=mask_d.ap())
        bf1sb = const.tile([P, FTI], f32, name="bf1sb", tag="bf1sb")
        nc.sync.dma_start(out=bf1sb, in_=bf1_d.ap())
        eps = const.tile([P, 1], f32, name="eps", tag="eps")
        nc.vector.memset(eps, 1e-5)
        zcol = const.tile([P, 1], f32, name="zcol", tag="zcol")
        nc.vector.memset(zcol, 0.0)
        cff2 = const.tile([P, 1], f32, name="cff2", tag="cff2")
        nc.vector.memset(cff2, 1.0 / (WS * WS))

        def load_vec(name):
            t = const.tile([P, D], f16, name=f"{name}_sb", tag=f"vb_{name}")
            nc.sync.dma_start(out=t, in_=bc(vec_d[name].ap()))
            return t

        # round-robin psum->sbuf drain across ACT/DVE
        rr_state = [0]

        def drain(out, in_):
            rr_state[0] ^= 1
            if rr_state[0]:
                nc.scalar.copy(out=out, in_=in_)
            else:
                nc.vector.tensor_copy(out=out, in_=in_)

        def layer_norm(lnp, xr, gb, bb, out):
            stats = lnp.tile([P, 2, 6], f32, name="stats", tag="stats",
                             bufs=2)
            nc.vector.bn_stats(out=stats[:, 0, :], in_=xr[:, 0:512])
            nc.vector.bn_stats(out=stats[:, 1, :], in_=xr[:, 512:1024])
            mv = lnp.tile([P, 2], f32, name="mv", tag="mv", bufs=2)
            nc.vector.bn_aggr(out=mv, in_=stats)
            lnv = lnp.tile([P, 1], f32, name="lnv", tag="lnv", bufs=2)
            nc.scalar.activation(out=lnv, in_=mv[:, 1:2], func=ACT.Ln,
                                 bias=eps, scale=1.0)
            rstd = lnp.tile([P, 1], f32, name="rstd", tag="rstd", bufs=2)
            nc.scalar.activation(out=rstd, in_=lnv, func=ACT.Exp, scale=-0.5)
            nc.vector.tensor_scalar(out=out, in0=xr, scalar1=mv[:, 0:1],
                                    scalar2=rstd, op0=ALU.subtract,
                                    op1=ALU.mult)
            nc.vector.tensor_mul(out=out, in0=out, in1=gb)
            nc.gpsimd.tensor_add(out=out, in0=out, in1=bb)

        def proj_kT(psum, w_t, src_t, dst, ntok):
            """dst[:, i, tok] += sum_j W[:, :, i-slice]^T src ([d_out, tok])."""
            for i in range(DTI):
                for ck in range(ntok // 1024):
                    ps = psum.tile([P, 1024], f32, name="ps_p", tag="ps_p",
                                   bufs=3)
                    for half in range(2):
                        sl = slice(ck * 1024 + half * 512,
                                   ck * 1024 + (half + 1) * 512)
                        for jp in range(NDP):
                            nc.tensor.matmul(
                                ps[:, half * 512:(half + 1) * 512],
                                lhsT=w_t[:, 2 * jp:2 * jp + 2,
                                         i * P:(i + 1) * P],
                                rhs=src_t[:, 2 * jp:2 * jp + 2, sl],
                                start=(jp == 0), stop=(jp == NDP - 1),
                                perf_mode=DR)
                    drain(dst[:, i, ck * 1024:(ck + 1) * 1024], ps)

        def proj_v(psum, w_t, src_t, dst, ntok):
            """dst[:, t, d] = (src^T W) per token tile ([tok, d_out])."""
            for t in range(ntok // P):
                ps = psum.tile([P, 1024], f32, name="ps_p", tag="ps_p",
                               bufs=3)
                for half in range(2):
                    for jp in range(NDP):
                        nc.tensor.matmul(
                            ps[:, half * 512:(half + 1) * 512],
                            lhsT=src_t[:, 2 * jp:2 * jp + 2,
                                       t * P:(t + 1) * P],
                            rhs=w_t[:, 2 * jp:2 * jp + 2,
                                    half * 512:(half + 1) * 512],
                            start=(jp == 0), stop=(jp == NDP - 1),
                            perf_mode=DR)
                drain(dst[:, t, :], ps)

        def attention(stk, tagp, qT, kT, v, resid, gb, bb, y_out, masked,
                      transpose_dst, transpose_res=None):
            """Full attention + add&norm (+ optional per-u transposes of the
            LN output into transpose_dst [P, DTI, NQ] fp8, with an fp8
            residual copy in transpose_res). For the masked (self) attention
            the first query tile's V-path runs in f16 to kill the
            early-causal-token error tail."""
            psum_s = stk.enter_context(tc.tile_pool(name=f"{tagp}ps_s",
                                                    bufs=2, space="PSUM"))
            psum_o = stk.enter_context(tc.tile_pool(name=f"{tagp}ps_o",
                                                    bufs=2, space="PSUM"))
            psum_d = stk.enter_context(tc.tile_pool(name=f"{tagp}ps_d",
                                                    bufs=1, space="PSUM"))
            psum_t = stk.enter_context(tc.tile_pool(name=f"{tagp}ps_t",
                                                    bufs=1, space="PSUM"))
            expp = stk.enter_context(tc.tile_pool(name=f"{tagp}expp",
                                                  bufs=2))
            lnp = stk.enter_context(tc.tile_pool(name=f"{tagp}lnp", bufs=2))
            xrp = stk.enter_context(tc.tile_pool(name=f"{tagp}xrp", bufs=3))
            e16 = v16 = None
            if masked:
                # f16 V for kv tiles 0,1: V01 = ykv01^T @ WV (true scale x32)
                v01p = stk.enter_context(tc.tile_pool(name=f"{tagp}v01p",
                                                      bufs=1))
                wv16 = v01p.tile([P, DTI, D], f16, name="wv16", tag="wv16")
                nc.sync.dma_start(out=wv16, in_=r3(wv16_d.ap()))
                ykv01 = v01p.tile([P, DTI, 2 * P], f16, name="ykv01",
                                  tag="ykv01")
                nc.sync.dma_start(out=ykv01, in_=r3(ykv01_d.ap()))
                v16 = v01p.tile([P, 2, D], f16, name="v16", tag="v16")
                for t in range(2):
                    ps = psum_o.tile([P, 1024], f32, name="po", tag="po")
                    for half in range(2):
                        for j in range(DTI):
                            nc.tensor.matmul(
                                ps[:, half * 512:(half + 1) * 512],
                                lhsT=ykv01[:, j, t * P:(t + 1) * P],
                                rhs=wv16[:, j, half * 512:(half + 1) * 512],
                                start=(j == 0), stop=(j == DTI - 1))
                    nc.scalar.activation(out=v16[:, t, :], in_=ps,
                                         func=ACT.Copy, scale=WS)
            for c in range(NCH):
                qsl = slice(c * 512, (c + 1) * 512)
                nvp = 4 * (c + 1) if masked else NKP   # visible pairs
                etiles = []
                for p in range(nvp):
                    e = expp.tile([P, 2, 512], f8, name=f"e{p}", tag=f"e{p}")
                    for sub in range(2):
                        t = 2 * p + sub
                        ps = psum_s.tile([P, 512], f32, name="ps_s",
                                         tag="ps_s")
                        for jp in range(NDP):
                            nc.tensor.matmul(
                                ps, lhsT=kT[:, 2 * jp:2 * jp + 2,
                                            t * P:(t + 1) * P],
                                rhs=qT[:, 2 * jp:2 * jp + 2, qsl],
                                start=(jp == 0), stop=(jp == NDP - 1),
                                perf_mode=DR)
                        nc.scalar.activation(out=e[:, sub, :], in_=ps,
                                             func=ACT.Exp, scale=EXP_SCALE)
                        if masked and c == 0 and p == 0:
                            if sub == 0:
                                e16 = expp.tile([P, 2, P], f16, name="e16",
                                                tag="e16")
                            nc.scalar.activation(out=e16[:, sub, :],
                                                 in_=ps[:, 0:P],
                                                 func=ACT.Exp,
                                                 scale=EXP_SCALE)
                    etiles.append(e)
                # diagonal-pair causal masks (Pool, SBUF-only)
                if masked:
                    for u4 in range(4):
                        u = c * 4 + u4
                        usl = slice(u4 * P, (u4 + 1) * P)
                        if u == 0:
                            nc.gpsimd.tensor_mul(out=e16, in0=e16, in1=mask8)
                        else:
                            nc.gpsimd.tensor_mul(out=etiles[u][:, :, usl],
                                                 in0=etiles[u][:, :, usl],
                                                 in1=mask8)
                # denominators -> [1, 512] -> DRAM -> [128, 4] -> reciprocal
                pd = psum_d.tile([32, 512], f32, name="pd", tag="pd")
                if masked:
                    for u4 in range(4):
                        u = c * 4 + u4
                        usl = slice(u4 * P, (u4 + 1) * P)
                        if u == 0:
                            for sub in range(2):
                                nc.tensor.matmul(pd[0:32, usl],
                                                 lhsT=ones16,
                                                 rhs=e16[:, sub, :],
                                                 start=(sub == 0),
                                                 stop=(sub == 1))
                            continue
                        for p in range(u + 1):
                            nc.tensor.matmul(
                                pd[0:32, usl], lhsT=ones8,
                                rhs=etiles[p][:, :, usl],
                                start=(p == 0), stop=(p == u), perf_mode=DR)
                else:
                    for p in range(NKP):
                        nc.tensor.matmul(
                            pd[0:32, :], lhsT=ones8, rhs=etiles[p],
                            start=(p == 0), stop=(p == NKP - 1),
                            perf_mode=DR)
                dsb = lnp.tile([1, 512], f32, name="dsb", tag="dsb")
                nc.scalar.mul(dsb, pd[0:1, :], WS)
                dscr = dramp.tile([512], f32, name="dscr",
                                  tag=f"{tagp}dscr{c}")
                nc.sync.dma_start(out=dscr, in_=dsb)
                dT = lnp.tile([P, 4], f32, name="dT", tag="dT")
                nc.sync.dma_start(out=dT,
                                  in_=dscr.rearrange("(a p) -> p a", p=P))
                recT = lnp.tile([P, 4], f32, name="recT", tag="recT")
                nc.vector.reciprocal(recT, dT)
                for u4 in range(4):
                    u = c * 4 + u4
                    usl = slice(u4 * P, (u4 + 1) * P)
                    nv = u + 1 if masked else NKP
                    po = psum_o.tile([P, 1024], f32, name="po", tag="po")
                    for half in range(2):
                        hsl = slice(half * 512, (half + 1) * 512)
                        if masked and u == 0:
                            for sub in range(2):
                                nc.tensor.matmul(po[:, hsl],
                                                 lhsT=e16[:, sub, :],
                                                 rhs=v16[:, sub, hsl],
                                                 start=(sub == 0),
                                                 stop=(sub == 1))
                            continue
                        for p in range(nv):
                            nc.tensor.matmul(
                                po[:, hsl], lhsT=etiles[p][:, :, usl],
                                rhs=v[:, 2 * p:2 * p + 2, hsl],
                                start=(p == 0), stop=(p == nv - 1),
                                perf_mode=DR)
                    xr = xrp.tile([P, D], f16, name="xr", tag="xr")
                    nc.scalar.activation(out=xr[:, 0:512], in_=po[:, 0:512],
                                         func=ACT.Copy,
                                         scale=recT[:, u4:u4 + 1])
                    nc.vector.tensor_scalar(out=xr[:, 512:1024],
                                            in0=po[:, 512:1024],
                                            scalar1=recT[:, u4:u4 + 1],
                                            scalar2=None, op0=ALU.mult)
                    nc.vector.tensor_add(out=xr, in0=xr, in1=resid[u])
                    layer_norm(lnp, xr, gb, bb, y_out[u])
                    if transpose_dst is not None:
                        for ip in range(4):
                            pt = psum_t.tile([P, 2, P], f16, name="pt",
                                             tag="pt")
                            for sub in range(2):
                                i = 2 * ip + sub
                                nc.tensor.transpose(
                                    pt[:, sub, :],
                                    in_=y_out[u][:, i * P:(i + 1) * P],
                                    identity=ident)
                            dsl = transpose_dst[:, 2 * ip:2 * ip + 2,
                                                u * P:(u + 1) * P]
                            drain(dsl, pt)
                            if transpose_res is not None:
                                nc.vector.tensor_tensor(
                                    out=transpose_res[:, 2 * ip:2 * ip + 2,
                                                      u * P:(u + 1) * P],
                                    in0=pt, in1=dsl, op=ALU.subtract)

        # Pool lifetime plan (left side is a LIFO stack of nested scopes,
        # right side pools are append-only and released together at the end):
        #   left:  [y1hp (B->C2) [zw2+y1T (A->C1) [kT/v/qT+yres (A->B)
        #          [ykv8/yq8/w1 (A)] [B attn pools]] [C1 psum]] [C2 attn
        #          pools]] [D pools]
        #   right: wf1p (C1->end), y2p (C2->end), wf2p (D->end)
        with ExitStack() as stAll:
            y1hp = stAll.enter_context(tc.tile_pool(name="y1hp", bufs=1))
            y1h = [y1hp.tile([P, D], f16, name=f"y1h{u}", tag=f"y1h{u}")
                   for u in range(QTI)]
            qkvp = stAll.enter_context(tc.tile_pool(name="qkvp", bufs=1))
            with ExitStack() as stBC1:
                zw2 = stBC1.enter_context(tc.tile_pool(name="zw2", bufs=1))
                z8 = zw2.tile([P, DTI, S], f8, name="z8", tag="z8")
                w2 = {n: zw2.tile([P, DTI, D], f8, name=n, tag=n)
                      for n in ["wk2", "wv2", "wq2"]}
                y1T = zw2.tile([P, DTI, NQ], f8, name="y1T", tag="y1T")
                kT = qkvp.tile([P, DTI, S], f8, name="kT", tag="kT")
                v = qkvp.tile([P, KTI, D], f8, name="v", tag="v")
                qT = qkvp.tile([P, DTI, NQ], f8, name="qT", tag="qT")
                yres = [qkvp.tile([P, D], f16, name=f"yres{u}",
                                  tag=f"yres{u}") for u in range(QTI)]
                with ExitStack() as stAB:
                    # ===== stage A: loads + self-attn K/V/Q projections =====
                    with ExitStack() as stA:
                        pA = stA.enter_context(tc.tile_pool(name="pA",
                                                            bufs=1))
                        psum_a = stA.enter_context(tc.tile_pool(
                            name="psA", bufs=3, space="PSUM"))
                        ykv8 = pA.tile([P, DTI, S], f8, name="ykv8",
                                       tag="ykv8")
                        nc.sync.dma_start(out=ykv8, in_=r3(ykv8_d.ap()))
                        wk1 = pA.tile([P, DTI, D], f8, name="wk1", tag="wk1")
                        nc.sync.dma_start(out=wk1, in_=r3(w_d["wk1"].ap()))
                        wv1 = pA.tile([P, DTI, D], f8, name="wv1", tag="wv1")
                        nc.sync.dma_start(out=wv1, in_=r3(w_d["wv1"].ap()))
                        yq8 = pA.tile([P, DTI, NQ], f8, name="yq8",
                                      tag="yq8")
                        nc.sync.dma_start(out=yq8, in_=r3(yq8_d.ap()))
                        wq1 = pA.tile([P, DTI, D], f8, name="wq1", tag="wq1")
                        nc.sync.dma_start(out=wq1, in_=r3(w_d["wq1"].ap()))
                        # prefetches (DMA queue order = consumption order)
                        for u in range(QTI):
                            nc.sync.dma_start(
                                out=yres[u],
                                in_=yres_d.ap()[u * P:(u + 1) * P, :])
                        g1b, be1b = load_vec("g1"), load_vec("be1")
                        nc.sync.dma_start(out=z8, in_=r3(z8_d.ap()))
                        for n in ["wk2", "wv2", "wq2"]:
                            nc.sync.dma_start(out=w2[n], in_=r3(w_d[n].ap()))
                        g2b, be2b = load_vec("g2"), load_vec("be2")

                        proj_kT(psum_a, wk1, ykv8, kT, S)
                        proj_v(psum_a, wv1, ykv8, v, S)
                        proj_kT(psum_a, wq1, yq8, qT, NQ)

                    # ===== stage B: self-attention + LN1 + y1 transpose =====
                    with ExitStack() as stB:
                        attention(stB, "sa_", qT, kT, v, yres, g1b, be1b,
                                  y1h, masked=True, transpose_dst=y1T)

                # ===== stage C1: cross-attn K/V/Q projections =====
                # (reuses the kT/v/qT tag rings - disjoint lifetimes)
                wf1p = tc.alloc_tile_pool(name="wf1p", bufs=1, side="right")
                kT2 = qkvp.tile([P, DTI, S], f8, name="kT2", tag="kT")
                v2 = qkvp.tile([P, KTI, D], f8, name="v2", tag="v")
                qT2 = qkvp.tile([P, DTI, NQ], f8, name="qT2", tag="qT")
                with ExitStack() as stC1:
                    psum_c = stC1.enter_context(tc.tile_pool(
                        name="psC", bufs=3, space="PSUM"))
                    proj_kT(psum_c, w2["wk2"], z8, kT2, S)
                    proj_v(psum_c, w2["wv2"], z8, v2, S)
                    proj_kT(psum_c, w2["wq2"], y1T, qT2, NQ)

            # ===== stage C2: cross-attention + LN2 + y2 transpose =====
            y2p = tc.alloc_tile_pool(name="y2p", bufs=1, side="right")
            y2h = [y2p.tile([P, D], f16, name=f"y2h{u}", tag=f"y2h{u}")
                   for u in range(QTI)]
            y2T = y2p.tile([P, DTI, NQ], f8, name="y2T", tag="y2T")
            y2Tr = y2p.tile([P, DTI, NQ], f8, name="y2Tr", tag="y2Tr")
            wf1x = wf1p.tile([P, 2 * DTI, DFF], f8, name="wf1x", tag="wf1x")
            nc.sync.dma_start(out=wf1x, in_=r3(wf1x_d.ap()))
            g3b, be3b = load_vec("g3"), load_vec("be3")
            with ExitStack() as stC2:
                attention(stC2, "ca_", qT2, kT2, v2, y1h, g2b, be2b, y2h,
                          masked=False, transpose_dst=y2T,
                          transpose_res=y2Tr)

        # ================= stage D: FFN + LN3 + output ======================
        # FFN1: 3x fp8 passes (w-exact x y2T8, plus w8 x y2T-residual);
        # FFN2: 2x (h8 x w2-exact via interleaved [w8, wr] pairs + bias pair).
        wf2p = tc.alloc_tile_pool(name="wf2p", bufs=1, side="right")
        with ExitStack() as stD:
            psum_f = stD.enter_context(tc.tile_pool(name="psD", bufs=1,
                                                    space="PSUM"))
            hp = stD.enter_context(tc.tile_pool(name="hp", bufs=1))
            ln3p = stD.enter_context(tc.tile_pool(name="ln3p", bufs=2))
            xr3p = stD.enter_context(tc.tile_pool(name="xr3p", bufs=3))
            outp = stD.enter_context(tc.tile_pool(name="outp", bufs=2))
            wf2x = wf2p.tile([P, 2 * FTI + 2, D], f8, name="wf2x",
                             tag="wf2x")
            nc.sync.dma_start(out=wf2x, in_=r3(wf2x_d.ap()))
            for c in range(NCH):
                csl = slice(c * 512, (c + 1) * 512)
                h8 = hp.tile([P, FTI, 512], f8, name="h8", tag="h8")
                for s in range(FTI):
                    ph = psum_f.tile([P, 512], f32, name="ph", tag="ph",
                                     bufs=3)
                    for j in range(DTI):
                        nc.tensor.matmul(
                            ph,
                            lhsT=wf1x[:, 2 * j:2 * j + 2, s * P:(s + 1) * P],
                            rhs=dup2(y2T[:, j, csl]),
                            start=(j == 0), stop=False, perf_mode=DR)
                    for a in range(NDP):
                        nc.tensor.matmul(
                            ph,
                            lhsT=pairstep2(wf1x, 4 * a,
                                           slice(s * P, (s + 1) * P)),
                            rhs=y2Tr[:, 2 * a:2 * a + 2, csl],
                            start=False, stop=(a == NDP - 1), perf_mode=DR)
                    if s % 2 == 0:
                        nc.scalar.activation(out=h8[:, s, :], in_=ph,
                                             func=ACT.Relu,
                                             bias=bf1sb[:, s:s + 1],
                                             scale=1.0)
                    else:
                        nc.vector.tensor_scalar(out=h8[:, s, :], in0=ph,
                                                scalar1=bf1sb[:, s:s + 1],
                                                scalar2=zcol, op0=ALU.add,
                                                op1=ALU.max)
                for u4 in range(4):
                    u = c * 4 + u4
                    usl = slice(u4 * P, (u4 + 1) * P)
                    pf = psum_f.tile([P, 1024], f32, name="pf", tag="pf",
                                     bufs=2)
                    for half in range(2):
                        hsl = slice(half * 512, (half + 1) * 512)
                        for f in range(FTI + 1):
                            lhsT = (dup2(h8[:, f, usl]) if f < FTI else e0t)
                            nc.tensor.matmul(
                                pf[:, hsl], lhsT=lhsT,
                                rhs=wf2x[:, 2 * f:2 * f + 2, hsl],
                                start=(f == 0), stop=(f == FTI),
                                perf_mode=DR)
                    xr = xr3p.tile([P, D], f16, name="xr3", tag="xr3")
                    nc.scalar.activation(out=xr[:, 0:512], in_=pf[:, 0:512],
                                         func=ACT.Copy, scale=1.0 / (WS * WS))
                    nc.vector.tensor_scalar(out=xr[:, 512:1024],
                                            in0=pf[:, 512:1024], scalar1=cff2,
                                            scalar2=None, op0=ALU.mult)
                    nc.vector.tensor_add(out=xr, in0=xr, in1=y2h[u])
                    y3 = outp.tile([P, D], f16, name="y3", tag="y3")
                    layer_norm(ln3p, xr, g3b, be3b, y3)
                    nc.sync.dma_start(out=out_d.ap()[u * P:(u + 1) * P, :],
                                      in_=y3)
        wf2p.release()
        y2p.release()
        wf1p.release()

    nc.compile()
    return nc


_CACHE = {}


def _get_nc():
    if "nc" not in _CACHE:
        _CACHE["nc"] = build_nc()
    return _CACHE["nc"]


def _q_indices(h):
    """Interleaved q-tile ownership: core-half h owns global tiles h, h+2..."""
    tiles = np.arange(h, 2 * QTI, 2)
    return (tiles[:, None] * P + np.arange(P)[None, :]).reshape(-1)


FP8 = ml_dtypes.float8_e4m3


def _prep_core(c, y, Z, shared):
    b, h = c // 2, c % 2
    qi = _q_indices(h)
    yb = y[b]
    tril = np.tril(np.ones((P, P), np.float32))
    mask = np.zeros((P, 2, P), np.float32)
    if h == 0:
        mask[:, 0, :] = tril.T          # [k, q]: visible iff k <= q
    else:
        mask[:, 0, :] = 1.0
        mask[:, 1, :] = tril.T
    m = {
        "yq8": np.ascontiguousarray(yb[qi].T).astype(FP8),
        "ykv8": np.ascontiguousarray(yb.T).astype(FP8),
        "z8": np.ascontiguousarray(Z[b].T).astype(FP8),
        "yres": yb[qi].astype(np.float16),
        "mask8": mask.astype(FP8),
        "ykv01": np.ascontiguousarray(yb[0:2 * P].T).astype(np.float16),
    }
    m.update(shared)
    return m


def _interleave_hilo(w32, ntile):
    """[ntile*128, M] f32 -> [2*ntile*128, M] fp8 with per-tile rows
    interleaved as (hi_tile, residual_tile)."""
    hi = w32.astype(FP8)
    lo = (w32 - hi.astype(np.float32)).astype(FP8)
    m = w32.shape[1]
    out = np.empty((ntile, 2, P, m), FP8)
    out[:, 0] = hi.reshape(ntile, P, m)
    out[:, 1] = lo.reshape(ntile, P, m)
    return out.reshape(2 * ntile * P, m)


def kernel(**inputs):
    inp = {k: np.asarray(v) for k, v in inputs.items()}
    y = inp["y"].astype(np.float32)
    Z = inp["Z"].astype(np.float32)
    ws = np.float32(WS)
    wf2x = np.zeros(((2 * FTI + 2) * P, D), FP8)
    wf2x[:2 * DFF] = _interleave_hilo(
        inp["W_ff2"].astype(np.float32) * ws, FTI)
    b2 = inp["b_ff2"].astype(np.float32) * ws
    b2hi = b2.astype(FP8)
    wf2x[2 * DFF] = b2hi
    wf2x[2 * DFF + 1] = (b2 - b2hi.astype(np.float32)).astype(FP8)
    shared = {
        "wq1": (inp["WQ1"].astype(np.float32) * ws).astype(FP8),
        "wk1": (inp["WK1"].astype(np.float32) * ws).astype(FP8),
        "wv1": (inp["WV1"].astype(np.float32) * ws).astype(FP8),
        "wq2": (inp["WQ2"].astype(np.float32) * ws).astype(FP8),
        "wk2": (inp["WK2"].astype(np.float32) * ws).astype(FP8),
        "wv2": (inp["WV2"].astype(np.float32) * ws).astype(FP8),
        "wv16": inp["WV1"].astype(np.float16),
        "wf1x": _interleave_hilo(inp["W_ff1"].astype(np.float32) * ws, DTI),
        "wf2x": wf2x,
        "bf1x32": np.ascontiguousarray(
            (inp["b_ff1"].astype(np.float32) * ws).reshape(FTI, P).T),
        "g1": inp["g1"].astype(np.float16),
        "be1": inp["be1"].astype(np.float16),
        "g2": inp["g2"].astype(np.float16),
        "be2": inp["be2"].astype(np.float16),
        "g3": inp["g3"].astype(np.float16),
        "be3": inp["be3"].astype(np.float16),
    }
    in_maps = [_prep_core(c, y, Z, shared) for c in range(N_CORES)]
    res = run_bass_kernel_spmd(_get_nc(), in_maps, list(range(N_CORES)))
    out = np.zeros((4, 2048, 1024), np.float32)
    for c in range(N_CORES):
        b, h = c // 2, c % 2
        out[b, _q_indices(h)] = res.results[c]["out16"].astype(np.float32)
    return out
